# revision 18
# baseline (speedup 1.0000x reference)
"""Trainium2 Bass kernel for nn_DRSolver (Douglas-Rachford QP solver).

Mathematical collapse
---------------------
In the reference, the Jacobian JF = [[A,0],[G,I]] and the Hessian
blockdiag(Q,0) are constant across batch and iterations, so the per-sample
QR/Cholesky factorizations are all identical and can be precomputed once on
the host.  The whole prox_g1 becomes an affine map

    y = P @ x + c,    P = Qn (Qn^T Md Qn)^-1 Qn^T   (96x96, constant)
                      c = C @ parms                 (per-sample, constant
                                                     across DR iterations)

With GAMMA=2 the DR update x' = x + clip(2y-x) - y simplifies: the +-1000
box bounds never bind for randn-scale data, and on the 32 slack rows

    x'[:64] = y[:64]
    x'[64:] = max(y_s, u) = u + relu(v),   u = x_s - y_s,  v = 2 y_s - x_s.

Relu-lifted state (the key device trick): iterate on z = [y_top; u; relu(v)]
(128 rows).  All three blocks are LINEAR in z of the previous step (the
matmul weights absorb x'_s = u + relu(v) by duplicating the 32 slack weight
rows), so each DR step is exactly

    psum[128, cols] = Wp @ parms + Wx @ z          (two matmuls, one bank)
    z' = max(psum, s_pp)                           (ONE VectorE tensor_scalar:
                                                    s_pp = -3e38 on rows 0:96
                                                    -> copy, 0 on rows 96:128
                                                    -> relu, fused evacuation)

Everything is fp16 (10-bit mantissa == tf32 precision at these magnitudes,
half DMA bytes).  8 cores, batch-sharded 512 samples/core, two 256-col
blocks pipelined per step.
"""

import numpy as np

import concourse.bass as bass
import concourse.tile as tile
import concourse.mybir as mybir
from concourse.bass_utils import run_bass_kernel_spmd

X_DIM, N_INEQ, N_EQ = 64, 32, 16
N = X_DIM + N_INEQ          # 96
M = N_EQ + N_INEQ           # 48
NP = X_DIM + N_EQ + N_INEQ  # 112 (parms dim)
NUM_STEPS = 10
BATCH = 4096
NCORES = 8
BPC = BATCH // NCORES       # 512 samples per core
COLB = 256                  # column block
PACK_M = 128                # psum rows: 64 y_top + 32 u + 32 v

F32 = mybir.dt.float32
F16 = mybir.dt.float16
NEG_BIG = -3.0e38


def _precompute(Q: np.ndarray, A: np.ndarray, G: np.ndarray) -> dict[str, np.ndarray]:
    """Host-side factorization collapse (float64, cast to f16)."""
    Qd, Ad, Gd = (m.astype(np.float64) for m in (Q, A, G))
    JF = np.zeros((M, N))
    JF[:N_EQ, :X_DIM] = Ad
    JF[N_EQ:, :X_DIM] = Gd
    JF[N_EQ:, X_DIM:] = np.eye(N_INEQ)
    Md = np.eye(N)
    Md[:X_DIM, :X_DIM] += Qd                      # gamma/2 * I + blockdiag(Q,0)
    Qc, _ = np.linalg.qr(JF.T, mode="complete")
    Qn = Qc[:, M:]                                # null-space basis of JF
    S = Qn.T @ Md @ Qn
    P = Qn @ np.linalg.solve(S, Qn.T)
    Z = JF.T @ np.linalg.solve(JF @ JF.T, np.eye(M))  # pinv(JF)
    C = np.zeros((N, NP))
    C[:, :X_DIM] = -P[:, :X_DIM]
    C[:, X_DIM:] = Z - P @ (Md @ Z)

    Es = np.eye(N)[X_DIM:]
    Ps, Cs = P[X_DIM:], C[X_DIM:]
    Wfull = np.concatenate([P[:X_DIM], Es - Ps, 2 * Ps - Es], 0)   # [128, 96]
    Wp = np.concatenate([C[:X_DIM], -Cs, 2 * Cs], 0)               # [128, 112]
    # x' = L z with L = [[I,0,0],[0,I,I]]; Wx = (Wfull L).T is Wfull.T with
    # the 32 slack rows duplicated.
    Wx = np.concatenate([Wfull.T, Wfull.T[X_DIM:N]], 0)            # [128, 128]
    # Combined stationary tile [128, 256]: cols 0:128 = Wp.T, cols 128:256 = Wx
    w = np.zeros((PACK_M, 2 * PACK_M), dtype=np.float64)
    w[:NP, :PACK_M] = Wp.T
    w[:, PACK_M:] = Wx
    return {"w": w.astype(np.float16)}


def _build_nc() -> bass.Bass:
    nc = bass.Bass()
    w_d = nc.dram_tensor("w", [PACK_M, 2 * PACK_M], F16, kind="ExternalInput")
    xt_d = nc.dram_tensor("xt", [N, BPC], F16, kind="ExternalInput")
    pt_d = nc.dram_tensor("pt", [NP, BPC], F16, kind="ExternalInput")
    yt_d = nc.dram_tensor("yt", [X_DIM, BPC], F16, kind="ExternalOutput")

    with tile.TileContext(nc) as tc:
        with (
            tc.tile_pool(name="sbuf", bufs=1) as cpool,
            tc.tile_pool(name="state", bufs=2) as spool,
            tc.tile_pool(name="psum", bufs=4, space="PSUM") as ppool,
            tc.tile_pool(name="warmps", bufs=1, space="PSUM") as wpool,
        ):
            w_sb = cpool.tile([PACK_M, 2 * PACK_M], F16, tag="w")
            xt = cpool.tile([N, BPC], F16, tag="xt")
            pt = cpool.tile([NP, BPC], F16, tag="pt")

            # HAM warm-up: the PE clock-gate releases only after ~3.4us of
            # sustained matmul activity; fill the input-DMA wait with dummy
            # matmuls on a scratch tile so the DR loop runs at 2.4 GHz.
            # GpSimd exits its preamble first, so it supplies the scratch
            # write the Tile dep-tracker requires with minimal delay.
            scr = cpool.tile([PACK_M, 3 * PACK_M], F16, tag="scr")
            nc.gpsimd.memset(scr[:], 0.0)
            wps = wpool.tile([PACK_M, 3 * PACK_M], F32, tag="warm")
            for _ in range(8):
                nc.tensor.matmul(wps[:], scr[:, :PACK_M], scr[:],
                                 start=True, stop=True)

            # Per-partition tensor_scalar operand: rows 0:96 -> max(x,-BIG)
            # (plain copy), rows 96:128 -> max(v,0) (relu).
            spp = cpool.tile([PACK_M, 1], F32, tag="spp")
            nc.vector.memset(spp[:N, :], NEG_BIG)
            nc.vector.memset(spp[N:, :], 0.0)

            # Input DMA, split across all three DGE paths so block 0's
            # operands land as early as possible.
            nc.sync.dma_start(w_sb[:], w_d[:])
            nc.scalar.dma_start(pt[:, :COLB], pt_d[:, :COLB])
            nc.sync.dma_start(xt[:, :COLB], xt_d[:, :COLB])
            nc.scalar.dma_start(pt[:, COLB:], pt_d[:, COLB:])
            nc.gpsimd.dma_start(xt[:, COLB:], xt_d[:, COLB:])

            wp = w_sb[:NP, :PACK_M]               # [112, 128] K=parms
            wx = w_sb[:, PACK_M:]                 # [128, 128] K=lifted state

            zprev = None

            for k in range(NUM_STEPS - 1):
                zn = spool.tile([PACK_M, 2 * COLB], F16, tag="zn")
                pys = []
                for j in range(2):
                    py = ppool.tile([PACK_M, COLB], F32, tag="pyu")
                    nc.tensor.matmul(py[:], wp, pt[:, bass.ts(j, COLB)],
                                     start=True, stop=False)
                    pys.append(py)
                for j in range(2):
                    if zprev is None:
                        nc.tensor.matmul(pys[j][:], w_sb[:N, PACK_M:],
                                         xt[:, bass.ts(j, COLB)],
                                         start=False, stop=True)
                    else:
                        nc.tensor.matmul(pys[j][:], wx,
                                         zprev[:, bass.ts(j, COLB)],
                                         start=False, stop=True)
                for j in range(2):
                    nc.vector.tensor_scalar(zn[:, bass.ts(j, COLB)], pys[j][:],
                                            spp[:], None, mybir.AluOpType.max)
                zprev = zn

            # Final step: only y[:64] is needed.
            yo = spool.tile([X_DIM, 2 * COLB], F16, tag="yo")
            for j in range(2):
                py = ppool.tile([X_DIM, COLB], F32, tag="pyu")
                nc.tensor.matmul(py[:], w_sb[:NP, :X_DIM],
                                 pt[:, bass.ts(j, COLB)], start=True, stop=False)
                nc.tensor.matmul(py[:], w_sb[:, PACK_M:PACK_M + X_DIM],
                                 zprev[:, bass.ts(j, COLB)],
                                 start=False, stop=True)
                sl = bass.ts(j, COLB)
                nc.vector.tensor_copy(yo[:, sl], py[:])
                if j == 0:
                    nc.sync.dma_start(yt_d[:, sl], yo[:, sl])
                else:
                    nc.scalar.dma_start(yt_d[:, sl], yo[:, sl])

    _legalize_waits(nc)
    return nc


# Barrier/teardown instructions that walrus handles specially; leave alone.
_WAIT_EXEMPT = {"InstEventSemaphore", "InstUnconditionalBranch", "InstCall"}


def _legalize_waits(nc: bass.Bass) -> None:
    """The TPB instruction structs carry a single sync-wait slot, and Tile's
    sem assignment can attach 2+ waits to one instruction (walrus then dies
    with 'Too many sync wait commands').  Fix up the final BIR: drop waits an
    earlier same-engine instruction already guaranteed, and hoist any
    remaining excess waits onto freshly inserted single-wait NoOps."""
    observed: dict[object, dict[int, int]] = {}
    cnt = 0
    for bb in nc.m.functions[0].blocks:
        insts = bb.instructions
        out: list = []
        for ins in insts:
            si = ins.sync_info
            tname = type(ins).__name__
            if si is not None and si.on_wait and tname not in _WAIT_EXEMPT:
                seen = observed.setdefault(ins.engine, {})
                kept = []
                for w in si.on_wait:
                    mono = (w.sync_type == "semaphore"
                            and w.wait_mode == "sem-ge-imm"
                            and w.wait_reg is None)
                    if mono and seen.get(w.id, -1) >= w.wait_value:
                        continue  # engine already waited at least this far
                    kept.append(w)
                    if mono:
                        seen[w.id] = max(seen.get(w.id, -1), w.wait_value)
                while len(kept) > 1:
                    w = kept.pop(0)
                    cnt += 1
                    nop = mybir.InstNoOp(name=f"waitnop-{cnt}", ins=[], outs=[])
                    nop.engine = ins.engine
                    nop.sync_info = mybir.SyncInfo(on_wait=[w], on_update=[])
                    nc.inst_map[nop.name] = nop
                    out.append(nop)
                si.on_wait = kept
            elif si is not None and si.on_wait:
                seen = observed.setdefault(ins.engine, {})
                for w in si.on_wait:
                    if (w.sync_type == "semaphore" and w.wait_mode == "sem-ge-imm"
                            and w.wait_reg is None):
                        seen[w.id] = max(seen.get(w.id, -1), w.wait_value)
            out.append(ins)
        if len(out) != len(insts):
            insts[:] = out


_NC_CACHE: bass.Bass | None = None

# Set by an external harness to enable NTFF tracing; harmless defaults.
TRACE = False
TRACE_DIR: str | None = None
LAST_RESULTS = None


def _get_nc() -> bass.Bass:
    global _NC_CACHE
    if _NC_CACHE is None:
        _NC_CACHE = _build_nc()
    return _NC_CACHE


def kernel(x: np.ndarray, parms: np.ndarray, Q: np.ndarray, A: np.ndarray,
           G: np.ndarray) -> np.ndarray:
    x = np.asarray(x, dtype=np.float32)
    parms = np.asarray(parms, dtype=np.float32)
    w = _precompute(np.asarray(Q), np.asarray(A), np.asarray(G))

    nc = _get_nc()
    in_maps = []
    for c in range(NCORES):
        lo, hi = c * BPC, (c + 1) * BPC
        in_maps.append({
            "xt": np.ascontiguousarray(x[lo:hi].T.astype(np.float16)),
            "pt": np.ascontiguousarray(parms[lo:hi].T.astype(np.float16)),
            **w,
        })
    global LAST_RESULTS
    kw = {}
    if TRACE:
        kw = {"trace": True, "tmpdir": TRACE_DIR}
    r = run_bass_kernel_spmd(nc, in_maps, list(range(NCORES)), **kw)
    LAST_RESULTS = r
    res = r.results
    out = np.empty((BATCH, X_DIM), dtype=np.float32)
    for c in range(NCORES):
        out[c * BPC:(c + 1) * BPC] = res[c]["yt"].T.astype(np.float32)
    return out


# revision 20
# speedup vs baseline: 1.0028x; 1.0028x over previous
"""Trainium2 Bass kernel for nn_DRSolver (Douglas-Rachford QP solver).

Mathematical collapse
---------------------
In the reference, the Jacobian JF = [[A,0],[G,I]] and the Hessian
blockdiag(Q,0) are constant across batch and iterations, so the per-sample
QR/Cholesky factorizations are all identical and can be precomputed once on
the host.  The whole prox_g1 becomes an affine map

    y = P @ x + c,    P = Qn (Qn^T Md Qn)^-1 Qn^T   (96x96, constant)
                      c = C @ parms                 (per-sample, constant
                                                     across DR iterations)

With GAMMA=2 the DR update x' = x + clip(2y-x) - y simplifies: the +-1000
box bounds never bind for randn-scale data, and on the 32 slack rows

    x'[:64] = y[:64]
    x'[64:] = max(y_s, u) = u + relu(v),   u = x_s - y_s,  v = 2 y_s - x_s.

Relu-lifted state (the key device trick): iterate on z = [y_top; u; relu(v)]
(128 rows).  All three blocks are LINEAR in z of the previous step (the
matmul weights absorb x'_s = u + relu(v) by duplicating the 32 slack weight
rows), so each DR step is exactly

    psum[128, cols] = Wp @ parms + Wx @ z          (two matmuls, one bank)
    z' = max(psum, s_pp)                           (ONE VectorE tensor_scalar:
                                                    s_pp = -3e38 on rows 0:96
                                                    -> copy, 0 on rows 96:128
                                                    -> relu, fused evacuation)

Everything is fp16 (10-bit mantissa == tf32 precision at these magnitudes,
half DMA bytes).  8 cores, batch-sharded 512 samples/core, two 256-col
blocks pipelined per step.
"""

import numpy as np

import concourse.bass as bass
import concourse.tile as tile
import concourse.mybir as mybir
from concourse.bass_utils import run_bass_kernel_spmd

X_DIM, N_INEQ, N_EQ = 64, 32, 16
N = X_DIM + N_INEQ          # 96
M = N_EQ + N_INEQ           # 48
NP = X_DIM + N_EQ + N_INEQ  # 112 (parms dim)
NUM_STEPS = 10
BATCH = 4096
NCORES = 8
BPC = BATCH // NCORES       # 512 samples per core
COLB = 256                  # column block
PACK_M = 128                # psum rows: 64 y_top + 32 u + 32 v

F32 = mybir.dt.float32
F16 = mybir.dt.float16
NEG_BIG = -3.0e38


def _precompute(Q: np.ndarray, A: np.ndarray, G: np.ndarray) -> dict[str, np.ndarray]:
    """Host-side factorization collapse (float64, cast to f16)."""
    Qd, Ad, Gd = (m.astype(np.float64) for m in (Q, A, G))
    JF = np.zeros((M, N))
    JF[:N_EQ, :X_DIM] = Ad
    JF[N_EQ:, :X_DIM] = Gd
    JF[N_EQ:, X_DIM:] = np.eye(N_INEQ)
    Md = np.eye(N)
    Md[:X_DIM, :X_DIM] += Qd                      # gamma/2 * I + blockdiag(Q,0)
    Qc, _ = np.linalg.qr(JF.T, mode="complete")
    Qn = Qc[:, M:]                                # null-space basis of JF
    S = Qn.T @ Md @ Qn
    P = Qn @ np.linalg.solve(S, Qn.T)
    Z = JF.T @ np.linalg.solve(JF @ JF.T, np.eye(M))  # pinv(JF)
    C = np.zeros((N, NP))
    C[:, :X_DIM] = -P[:, :X_DIM]
    C[:, X_DIM:] = Z - P @ (Md @ Z)

    Es = np.eye(N)[X_DIM:]
    Ps, Cs = P[X_DIM:], C[X_DIM:]
    Wfull = np.concatenate([P[:X_DIM], Es - Ps, 2 * Ps - Es], 0)   # [128, 96]
    Wp = np.concatenate([C[:X_DIM], -Cs, 2 * Cs], 0)               # [128, 112]
    # x' = L z with L = [[I,0,0],[0,I,I]]; Wx = (Wfull L).T is Wfull.T with
    # the 32 slack rows duplicated.
    Wx = np.concatenate([Wfull.T, Wfull.T[X_DIM:N]], 0)            # [128, 128]
    # Combined stationary tile [128, 256]: cols 0:128 = Wp.T, cols 128:256 = Wx
    w = np.zeros((PACK_M, 2 * PACK_M), dtype=np.float64)
    w[:NP, :PACK_M] = Wp.T
    w[:, PACK_M:] = Wx
    return {"w": w.astype(np.float16)}


def _build_nc() -> bass.Bass:
    nc = bass.Bass()
    w_d = nc.dram_tensor("w", [PACK_M, 2 * PACK_M], F16, kind="ExternalInput")
    xt_d = nc.dram_tensor("xt", [N, BPC], F16, kind="ExternalInput")
    pt_d = nc.dram_tensor("pt", [NP, BPC], F16, kind="ExternalInput")
    yt_d = nc.dram_tensor("yt", [X_DIM, BPC], F16, kind="ExternalOutput")

    with tile.TileContext(nc) as tc:
        with (
            tc.tile_pool(name="sbuf", bufs=1) as cpool,
            tc.tile_pool(name="state", bufs=2) as spool,
            tc.tile_pool(name="psum", bufs=4, space="PSUM") as ppool,
            tc.tile_pool(name="warmps", bufs=1, space="PSUM") as wpool,
        ):
            w_sb = cpool.tile([PACK_M, 2 * PACK_M], F16, tag="w")
            xt = cpool.tile([N, BPC], F16, tag="xt")
            pt = cpool.tile([NP, BPC], F16, tag="pt")

            # HAM warm-up: the PE clock-gate releases only after ~3.4us of
            # sustained matmul activity; fill the input-DMA wait with dummy
            # matmuls on a scratch tile so the DR loop runs at 2.4 GHz.
            # GpSimd exits its preamble first, so it supplies the scratch
            # write the Tile dep-tracker requires with minimal delay.
            scr = cpool.tile([PACK_M, 3 * PACK_M], F16, tag="scr")
            nc.gpsimd.memset(scr[:], 0.0)
            wps = wpool.tile([PACK_M, 3 * PACK_M], F32, tag="warm")
            for _ in range(8):
                nc.tensor.matmul(wps[:], scr[:, :PACK_M], scr[:],
                                 start=True, stop=True)

            # Per-partition tensor_scalar operand: rows 0:96 -> max(x,-BIG)
            # (plain copy), rows 96:128 -> max(v,0) (relu).
            spp = cpool.tile([PACK_M, 1], F32, tag="spp")
            nc.vector.memset(spp[:N, :], NEG_BIG)
            nc.vector.memset(spp[N:, :], 0.0)

            # Input DMA, split across all three DGE paths so block 0's
            # operands land as early as possible.
            nc.sync.dma_start(w_sb[:], w_d[:])
            nc.scalar.dma_start(pt[:, :COLB], pt_d[:, :COLB])
            nc.sync.dma_start(xt[:, :COLB], xt_d[:, :COLB])
            nc.scalar.dma_start(pt[:, COLB:], pt_d[:, COLB:])
            nc.gpsimd.dma_start(xt[:, COLB:], xt_d[:, COLB:])

            wp = w_sb[:NP, :PACK_M]               # [112, 128] K=parms
            wx = w_sb[:, PACK_M:]                 # [128, 128] K=lifted state

            zprev = None

            for k in range(NUM_STEPS - 1):
                zn = spool.tile([PACK_M, 2 * COLB], F16, tag="zn")
                pys = []
                for j in range(2):
                    py = ppool.tile([PACK_M, COLB], F32, tag="pyu")
                    nc.tensor.matmul(py[:], wp, pt[:, bass.ts(j, COLB)],
                                     start=True, stop=False)
                    pys.append(py)
                for j in range(2):
                    if zprev is None:
                        nc.tensor.matmul(pys[j][:], w_sb[:N, PACK_M:],
                                         xt[:, bass.ts(j, COLB)],
                                         start=False, stop=True)
                    else:
                        nc.tensor.matmul(pys[j][:], wx,
                                         zprev[:, bass.ts(j, COLB)],
                                         start=False, stop=True)
                for j in range(2):
                    nc.vector.tensor_scalar(zn[:, bass.ts(j, COLB)], pys[j][:],
                                            spp[:], None, mybir.AluOpType.max)
                zprev = zn

            # Final step: only y[:64] is needed.
            yo = spool.tile([X_DIM, 2 * COLB], F16, tag="yo")
            for j in range(2):
                py = ppool.tile([X_DIM, COLB], F32, tag="pyu")
                nc.tensor.matmul(py[:], w_sb[:NP, :X_DIM],
                                 pt[:, bass.ts(j, COLB)], start=True, stop=False)
                nc.tensor.matmul(py[:], w_sb[:, PACK_M:PACK_M + X_DIM],
                                 zprev[:, bass.ts(j, COLB)],
                                 start=False, stop=True)
                sl = bass.ts(j, COLB)
                nc.vector.tensor_copy(yo[:, sl], py[:])
                if j == 0:
                    nc.sync.dma_start(yt_d[:, sl], yo[:, sl])
                else:
                    nc.scalar.dma_start(yt_d[:, sl], yo[:, sl])

    _legalize_waits(nc)
    return nc


# Barrier/teardown instructions that walrus handles specially; leave alone.
_WAIT_EXEMPT = {"InstEventSemaphore", "InstUnconditionalBranch", "InstCall"}


def _legalize_waits(nc: bass.Bass) -> None:
    """The TPB instruction structs carry a single sync-wait slot, and Tile's
    sem assignment can attach 2+ waits to one instruction (walrus then dies
    with 'Too many sync wait commands').  Fix up the final BIR: drop waits an
    earlier same-engine instruction already guaranteed, and hoist any
    remaining excess waits onto freshly inserted single-wait NoOps."""
    observed: dict[object, dict[int, int]] = {}
    cnt = 0
    for bb in nc.m.functions[0].blocks:
        insts = bb.instructions
        out: list = []
        for ins in insts:
            si = ins.sync_info
            tname = type(ins).__name__
            if si is not None and si.on_wait and tname not in _WAIT_EXEMPT:
                seen = observed.setdefault(ins.engine, {})
                kept = []
                for w in si.on_wait:
                    mono = (w.sync_type == "semaphore"
                            and w.wait_mode == "sem-ge-imm"
                            and w.wait_reg is None)
                    if mono and seen.get(w.id, -1) >= w.wait_value:
                        continue  # engine already waited at least this far
                    kept.append(w)
                    if mono:
                        seen[w.id] = max(seen.get(w.id, -1), w.wait_value)
                while len(kept) > 1:
                    w = kept.pop(0)
                    cnt += 1
                    nop = mybir.InstNoOp(name=f"waitnop-{cnt}", ins=[], outs=[])
                    nop.engine = ins.engine
                    nop.sync_info = mybir.SyncInfo(on_wait=[w], on_update=[])
                    nc.inst_map[nop.name] = nop
                    out.append(nop)
                si.on_wait = kept
            elif si is not None and si.on_wait:
                seen = observed.setdefault(ins.engine, {})
                for w in si.on_wait:
                    if (w.sync_type == "semaphore" and w.wait_mode == "sem-ge-imm"
                            and w.wait_reg is None):
                        seen[w.id] = max(seen.get(w.id, -1), w.wait_value)
            out.append(ins)
        if len(out) != len(insts):
            insts[:] = out


_NC_CACHE: bass.Bass | None = None

# Set by an external harness to enable NTFF tracing; harmless defaults.
TRACE = False
TRACE_DIR: str | None = None
LAST_RESULTS = None


def _get_nc() -> bass.Bass:
    global _NC_CACHE
    if _NC_CACHE is None:
        _NC_CACHE = _build_nc()
    return _NC_CACHE


def kernel(x: np.ndarray, parms: np.ndarray, Q: np.ndarray, A: np.ndarray,
           G: np.ndarray) -> np.ndarray:
    x = np.asarray(x, dtype=np.float32)
    parms = np.asarray(parms, dtype=np.float32)
    w = _precompute(np.asarray(Q), np.asarray(A), np.asarray(G))

    nc = _get_nc()
    in_maps = []
    for c in range(NCORES):
        lo, hi = c * BPC, (c + 1) * BPC
        in_maps.append({
            "xt": np.ascontiguousarray(x[lo:hi].T.astype(np.float16)),
            "pt": np.ascontiguousarray(parms[lo:hi].T.astype(np.float16)),
            **w,
        })
    global LAST_RESULTS
    kw = {}
    if TRACE:
        kw = {"trace": True, "tmpdir": TRACE_DIR}
    r = run_bass_kernel_spmd(nc, in_maps, list(range(NCORES)), **kw)
    LAST_RESULTS = r
    res = r.results
    out = np.empty((BATCH, X_DIM), dtype=np.float32)
    for c in range(NCORES):
        out[c * BPC:(c + 1) * BPC] = res[c]["yt"].T.astype(np.float32)
    return out
